# revision 6
# baseline (speedup 1.0000x reference)
"""MoLoRA (top-2 of 8 LoRA experts, dense compute) Trainium2 Bass kernel.

Math (matches the jax reference exactly in structure):
    xs [T,1024], Wg [1024,8], A_flat [1024,128] (j = e*16+r), B_flat [128,1024]
    logits = xs @ Wg                      (fp32, exact -> exact top-2 choice)
    cw     = dense top-2 softmax weights  [T,8]
    H^T    = A_flat^T @ xs^T              (f32r, feature-major [128 j, T])
    HW^T   = H^T * cw^T(expanded)         (f32r)
    out    = (HW^T)^T-matmul B_flat       (f32r, token-major [T,1024])

Sharding: pure data-parallel over tokens; 8 cores x 4096 tokens.
Per core: 8 supertiles x 512 tokens; 4 subtiles x 128 tokens each.
x is transposed on-chip via PE transposes (fp32); the gate runs in true
fp32 off the transposed x, the expert matmuls run in f32r (fast PE mode,
~1.5e-4 matmul rel-err measured on HW).
"""

import numpy as np

import concourse.bacc as bacc
import concourse.mybir as mybir
import concourse.tile as tile
from concourse.bass_utils import run_bass_kernel_spmd
from concourse.masks import make_identity

F32 = mybir.dt.float32
F32R = mybir.dt.float32r

N_CORES = 8
D = 1024
E = 8
R = 16
J = E * R  # 128
T_FULL = 4 * 8192
T_SH = T_FULL // N_CORES  # 4096
P = 128
NCH = D // P  # 8 contraction chunks
N_SUPER = T_SH // 512  # 8 supertiles of 512 tokens
AX = mybir.AxisListType.X
OP = mybir.AluOpType


def build(n_super=N_SUPER):
    t_sh = n_super * 512
    nc = bacc.Bacc("TRN2", target_bir_lowering=False, debug=False)

    x_d = nc.declare_dram_parameter("x", [t_sh, D], F32, isOutput=False)
    wg_d = nc.declare_dram_parameter("wg", [P, NCH, E], F32, isOutput=False)
    a_d = nc.declare_dram_parameter("a", [P, NCH, J], F32, isOutput=False)
    b_d = nc.declare_dram_parameter("b", [P, D], F32, isOutput=False)
    out_d = nc.declare_dram_parameter("out", [t_sh, D], F32, isOutput=True)

    x_v = x_d[:].rearrange("(s c p) d -> s p c d", p=P, c=4)
    out_v = out_d[:].rearrange("(s c p) d -> s p c d", p=P, c=4)

    with tile.TileContext(nc) as tc:
        with (
            tc.tile_pool(name="consts", bufs=1) as consts,
            tc.tile_pool(name="xp", bufs=2) as xp,
            tc.tile_pool(name="xt", bufs=2) as xtp,
            tc.tile_pool(name="xtr", bufs=2) as xtrp,
            tc.tile_pool(name="cwp", bufs=2) as cwp,
            tc.tile_pool(name="hwp", bufs=2) as hwp,
            tc.tile_pool(name="osb", bufs=2) as osb,
            tc.tile_pool(name="tp", bufs=3, space="PSUM") as tp,
            tc.tile_pool(name="gps", bufs=2, space="PSUM") as gps,
            tc.tile_pool(name="hps", bufs=1, space="PSUM") as hps,
            tc.tile_pool(name="ops", bufs=2, space="PSUM") as ops,
        ):
            ident = consts.tile([P, P], F32)
            make_identity(nc, ident[:])
            wg_sb = consts.tile([P, NCH, E], F32)
            a_sb = consts.tile([P, NCH, J], F32)
            b_sb = consts.tile([P, D], F32)
            nc.sync.dma_start(wg_sb[:], wg_d[:])
            nc.sync.dma_start(a_sb[:], a_d[:])
            nc.sync.dma_start(b_sb[:], b_d[:])
            # round expert weights to f32r once
            a_r = consts.tile([P, NCH, J], F32R)
            b_r = consts.tile([P, D], F32R)
            nc.vector.tensor_copy(a_r[:], a_sb[:])
            nc.vector.tensor_copy(b_r[:], b_sb[:])

            for s in range(n_super):
                x_sb = xp.tile([P, 4, D], F32)
                nc.sync.dma_start(x_sb[:], x_v[s])

                gate_ps = gps.tile([P, 4, E], F32)
                h_ps = hps.tile([P, 512], F32)
                xt_fs = []
                xt_rs = []
                for pair in range(2):
                    xt_r = xtrp.tile([P, NCH, 256], F32R)
                    xt_rs.append(xt_r)
                    for ci in range(2):
                        c = pair * 2 + ci
                        xt_f = xtp.tile([P, NCH, P], F32)
                        xt_fs.append(xt_f)
                        for g in range(2):
                            tpt = tp.tile([P, 4, P], F32, tag="tp")
                            for k in range(4):
                                kk = g * 4 + k
                                nc.tensor.transpose(
                                    tpt[:, k, :],
                                    x_sb[:, c, kk * P:(kk + 1) * P],
                                    ident[:],
                                )
                            nc.vector.tensor_copy(xt_f[:, g * 4:(g + 1) * 4, :], tpt[:])
                        # round to f32r for the expert matmuls (gpsimd is idle)
                        nc.gpsimd.tensor_copy(
                            xt_r[:, :, ci * P:(ci + 1) * P], xt_f[:]
                        )
                        # gate: true-fp32 matmuls, N=8 (stationary = xT chunk)
                        for k in range(NCH):
                            nc.tensor.matmul(
                                gate_ps[:, c, :],
                                xt_f[:, k, :],
                                wg_sb[:, k, :],
                                start=(k == 0),
                                stop=(k == NCH - 1),
                            )
                    # H^T for this pair: [128 j, 256 tok], f32r N=256
                    for k in range(NCH):
                        nc.tensor.matmul(
                            h_ps[:, pair * 256:(pair + 1) * 256],
                            a_r[:, k, :],
                            xt_r[:, k, :],
                            start=(k == 0),
                            stop=(k == NCH - 1),
                        )

                # top-2 softmax -> dense combine weights, batched over 4 subtiles
                m1 = cwp.tile([P, 4], F32)
                m2 = cwp.tile([P, 4], F32)
                d21 = cwp.tile([P, 4], F32)
                w1 = cwp.tile([P, 4], F32)
                w2 = cwp.tile([P, 4], F32)
                eq1 = cwp.tile([P, 4, E], F32)
                msk = cwp.tile([P, 4, E], F32)
                eq2 = cwp.tile([P, 4, E], F32)
                cw = cwp.tile([P, 4, E], F32)
                nc.vector.tensor_reduce(m1[:], gate_ps[:], AX, OP.max)
                nc.vector.tensor_tensor(
                    eq1[:], gate_ps[:],
                    m1[:].unsqueeze(2).broadcast_to((P, 4, E)), OP.is_equal,
                )
                nc.vector.scalar_tensor_tensor(
                    msk[:], eq1[:], -1e30, gate_ps[:], OP.mult, OP.add
                )
                nc.vector.tensor_reduce(m2[:], msk[:], AX, OP.max)
                nc.vector.tensor_tensor(d21[:], m2[:], m1[:], OP.subtract)
                nc.scalar.activation(
                    w1[:], d21[:], mybir.ActivationFunctionType.Sigmoid, scale=-1.0
                )
                nc.scalar.activation(
                    w2[:], d21[:], mybir.ActivationFunctionType.Sigmoid
                )
                nc.vector.tensor_tensor(
                    eq2[:], msk[:],
                    m2[:].unsqueeze(2).broadcast_to((P, 4, E)), OP.is_equal,
                )
                nc.vector.tensor_tensor(
                    cw[:], eq1[:],
                    w1[:].unsqueeze(2).broadcast_to((P, 4, E)), OP.mult,
                )
                nc.vector.tensor_tensor(
                    eq2[:], eq2[:],
                    w2[:].unsqueeze(2).broadcast_to((P, 4, E)), OP.mult,
                )
                nc.vector.tensor_tensor(cw[:], cw[:], eq2[:], OP.add)

                # expand cw along R, transpose to feature-major, apply, project
                o_sb = osb.tile([P, 4, D], F32, tag="osb")
                for pair in range(2):
                    cwt_sb = cwp.tile([P, 2, P], F32, tag="cwt")
                    for ci in range(2):
                        c = pair * 2 + ci
                        cw_exp = cwp.tile([P, E, R], F32, tag="cwe")
                        nc.vector.tensor_copy(
                            cw_exp[:],
                            cw[:, c, :].unsqueeze(2).broadcast_to((P, E, R)),
                        )
                        cwt_ps = tp.tile([P, P], F32, tag="tp")
                        nc.tensor.transpose(
                            cwt_ps[:], cw_exp[:].rearrange("p e r -> p (e r)"),
                            ident[:],
                        )
                        nc.vector.tensor_copy(cwt_sb[:, ci, :], cwt_ps[:])
                    hw_sb = hwp.tile([P, 2, P], F32R)
                    nc.vector.tensor_tensor(
                        hw_sb[:].rearrange("p a b -> p (a b)"),
                        h_ps[:, pair * 256:(pair + 1) * 256],
                        cwt_sb[:].rearrange("p a b -> p (a b)"),
                        OP.mult,
                    )
                    for ci in range(2):
                        c = pair * 2 + ci
                        for h in range(2):
                            o_ps = ops.tile([P, 512], F32, tag="ops")
                            nc.tensor.matmul(
                                o_ps[:],
                                hw_sb[:, ci, :],
                                b_r[:, h * 512:(h + 1) * 512],
                                start=True,
                                stop=True,
                            )
                            nc.scalar.copy(
                                o_sb[:, c, h * 512:(h + 1) * 512], o_ps[:]
                            )
                nc.sync.dma_start(out_v[s], o_sb[:])

    nc.finalize()
    return nc


_NC_CACHE = {}


def _get_nc(n_super=N_SUPER):
    if n_super not in _NC_CACHE:
        _NC_CACHE[n_super] = build(n_super)
    return _NC_CACHE[n_super]


def _prep_weights(Wg, A, B):
    # wg[p, c, e] = Wg[c*128+p, e]
    wg = np.ascontiguousarray(
        Wg.reshape(NCH, P, E).transpose(1, 0, 2)
    ).astype(np.float32)
    # A_flat[d, e*R+r] = A[e, d, r];  a[p, c, j] = A_flat[c*128+p, j]
    a_flat = A.transpose(1, 0, 2).reshape(D, J)
    a = np.ascontiguousarray(
        a_flat.reshape(NCH, P, J).transpose(1, 0, 2)
    ).astype(np.float32)
    # B_flat[j, d] = B[j//R, j%R, d]
    b = np.ascontiguousarray(B.reshape(J, D)).astype(np.float32)
    return wg, a, b


def kernel(x, Wg, A, B):
    x = np.asarray(x, dtype=np.float32)
    orig_shape = x.shape
    xs = np.ascontiguousarray(x.reshape(-1, D))
    assert xs.shape[0] == T_FULL
    wg, a, b = _prep_weights(np.asarray(Wg, np.float32),
                             np.asarray(A, np.float32),
                             np.asarray(B, np.float32))

    nc = _get_nc()
    shards = np.split(xs, N_CORES, axis=0)
    in_maps = [
        {"x": np.ascontiguousarray(sh), "wg": wg, "a": a, "b": b}
        for sh in shards
    ]
    res = run_bass_kernel_spmd(nc, in_maps, list(range(N_CORES)))
    out = np.concatenate([r["out"] for r in res.results], axis=0)
    return out.reshape(orig_shape)


# revision 38
# speedup vs baseline: 194.4239x; 194.4239x over previous
"""MoLoRA (top-2 of 8 LoRA experts, dense compute) Trainium2 Bass kernel.

Math (matches the jax reference exactly in structure):
    xs [T,1024], Wg [1024,8], A_flat [1024,128] (j = e*16+r), B_flat [128,1024]
    logits = xs @ Wg                      (fp32, exact -> exact top-2 choice)
    cw     = dense top-2 softmax weights  [T,8]
    H^T    = A_flat^T @ xs^T              (f32r, feature-major [128 j, T])
    HW^T   = H^T * cw^T(expanded)         (f32r)
    out    = (HW^T)^T-matmul B_flat       (f32r, token-major [T,1024])

Sharding: pure data-parallel over tokens; 8 cores x 4096 tokens.
Per core: 16 pairs x 256 tokens (2 subtiles x 128).  x is transposed
on-chip via PE transposes (fp32); the gate runs in true fp32 off the
transposed x, the expert matmuls run in f32r (fast PE mode, ~1.5e-4
matmul rel-err measured on HW).
"""

import numpy as np

import concourse.bacc as bacc
import concourse.mybir as mybir
import concourse.tile as tile
from concourse.bass_utils import run_bass_kernel_spmd
from concourse.masks import make_identity

F32 = mybir.dt.float32
F32R = mybir.dt.float32r

N_CORES = 8
D = 1024
E = 8
R = 16
J = E * R  # 128
T_FULL = 4 * 8192
T_SH = T_FULL // N_CORES  # 4096
P = 128
NCH = D // P  # 8 contraction chunks
N_PAIR = T_SH // 256  # 16 pairs of 256 tokens
N_SUPER = N_PAIR // 2  # kept for compat with callers
AX = mybir.AxisListType.X
OP = mybir.AluOpType


def build(n_super=N_SUPER, n_reps=1):
    t_sh = n_super * 512
    nc = bacc.Bacc("TRN2", target_bir_lowering=False, debug=False)

    x_d = nc.declare_dram_parameter("x", [t_sh, D], F32, isOutput=False)
    wg_d = nc.declare_dram_parameter("wg", [P, NCH, E], F32, isOutput=False)
    a_d = nc.declare_dram_parameter("a", [P, NCH, J], F32, isOutput=False)
    b_d = nc.declare_dram_parameter("b", [P, D], F32, isOutput=False)
    out_d = nc.declare_dram_parameter("out", [t_sh, D], F32, isOutput=True)

    # s = supertile, c = subtile (4 of 128 tokens), p = token within subtile
    x_v = x_d[:].rearrange("(s c p) d -> s p c d", p=P, c=4)
    out_v = out_d[:].rearrange("(s c p) d -> s p c d", p=P, c=4)

    with tile.TileContext(nc) as tc:
        with (
            tc.tile_pool(name="consts", bufs=1) as consts,
            tc.tile_pool(name="xp", bufs=4) as xp,
            tc.tile_pool(name="xt", bufs=3) as xtp,
            tc.tile_pool(name="xtr", bufs=2) as xtrp,
            tc.tile_pool(name="cwp", bufs=3) as cwp,
            tc.tile_pool(name="hwp", bufs=3) as hwp,
            tc.tile_pool(name="osb", bufs=3) as osb,
            tc.tile_pool(name="tp", bufs=2, space="PSUM") as tp,
            tc.tile_pool(name="gps", bufs=2, space="PSUM") as gps,
            tc.tile_pool(name="hps", bufs=2, space="PSUM") as hps,
            tc.tile_pool(name="ops", bufs=2, space="PSUM") as ops,
        ):
            ident = consts.tile([P, P], F32)
            make_identity(nc, ident[:])
            wg_sb = consts.tile([P, NCH, E], F32)
            a_sb = consts.tile([P, NCH, J], F32)
            b_sb = consts.tile([P, D], F32)
            nc.sync.dma_start(wg_sb[:], wg_d[:])
            nc.sync.dma_start(a_sb[:], a_d[:])
            nc.sync.dma_start(b_sb[:], b_d[:])
            # round expert weights to f32r once
            a_r = consts.tile([P, NCH, J], F32R)
            b_r = consts.tile([P, D], F32R)
            nc.vector.tensor_copy(a_r[:], a_sb[:])
            nc.vector.tensor_copy(b_r[:], b_sb[:])

            for s in [t % n_super for t in range(n_super * n_reps)]:
                x_sb = xp.tile([P, 4, D], F32)
                # split the load per subtile so transposes start at 512 KiB
                for c in range(4):
                    nc.sync.dma_start(x_sb[:, c], x_v[s, :, c])

                gate_ps = gps.tile([P, 4, E], F32)
                h_ps = hps.tile([P, 512], F32)
                xt_r = xtrp.tile([P, NCH, 512], F32R)
                for c in range(4):
                    xt_fg0 = xtp.tile([P, 4, P], F32, tag="xt0")
                    xt_fg1 = xtp.tile([P, 4, P], F32, tag="xt1")
                    xt_fg = [xt_fg0, xt_fg1]
                    for g in range(2):
                        tpt = tp.tile([P, 4, P], F32, tag="tp")
                        for k in range(4):
                            kk = g * 4 + k
                            nc.tensor.transpose(
                                tpt[:, k, :],
                                x_sb[:, c, kk * P:(kk + 1) * P],
                                ident[:],
                            )
                        nc.vector.tensor_copy(xt_fg[g][:], tpt[:])
                        # round to f32r for the expert matmuls; split DVE/Pool
                        dst = xt_r[:, g * 4:(g + 1) * 4, c * P:(c + 1) * P]
                        if c < 2:
                            nc.vector.tensor_copy(dst, xt_fg[g][:])
                        else:
                            nc.gpsimd.tensor_copy(dst, xt_fg[g][:])
                        # gate: true-fp32 matmuls, N=8 (stationary = xT chunk)
                        for k in range(4):
                            nc.tensor.matmul(
                                gate_ps[:, c, :],
                                xt_fg[g][:, k, :],
                                wg_sb[:, g * 4 + k, :],
                                start=(g == 0 and k == 0),
                                stop=(g == 1 and k == 3),
                            )
                # H^T for this supertile: [128 j, 512 tok], f32r N=512
                for k in range(NCH):
                    nc.tensor.matmul(
                        h_ps[:],
                        a_r[:, k, :],
                        xt_r[:, k, :],
                        start=(k == 0),
                        stop=(k == NCH - 1),
                    )

                # top-2 softmax -> dense combine weights, batched over 4 subtiles
                m1 = cwp.tile([P, 4], F32)
                m2 = cwp.tile([P, 4], F32)
                d21 = cwp.tile([P, 4], F32)
                w1 = cwp.tile([P, 4], F32)
                w2 = cwp.tile([P, 4], F32)
                eq1 = cwp.tile([P, 4, E], F32)
                msk = cwp.tile([P, 4, E], F32)
                eq2 = cwp.tile([P, 4, E], F32)
                cw = cwp.tile([P, 4, E], F32)
                nc.vector.tensor_reduce(m1[:], gate_ps[:], AX, OP.max)
                nc.vector.tensor_tensor(
                    eq1[:], gate_ps[:],
                    m1[:].unsqueeze(2).broadcast_to((P, 4, E)), OP.is_equal,
                )
                nc.vector.scalar_tensor_tensor(
                    msk[:], eq1[:], -1e30, gate_ps[:], OP.mult, OP.add
                )
                nc.vector.tensor_reduce(m2[:], msk[:], AX, OP.max)
                nc.vector.tensor_tensor(d21[:], m2[:], m1[:], OP.subtract)
                nc.scalar.activation(
                    w1[:], d21[:], mybir.ActivationFunctionType.Sigmoid, scale=-1.0
                )
                nc.scalar.activation(
                    w2[:], d21[:], mybir.ActivationFunctionType.Sigmoid
                )
                nc.vector.tensor_tensor(
                    eq2[:], msk[:],
                    m2[:].unsqueeze(2).broadcast_to((P, 4, E)), OP.is_equal,
                )
                nc.vector.tensor_tensor(
                    cw[:], eq1[:],
                    w1[:].unsqueeze(2).broadcast_to((P, 4, E)), OP.mult,
                )
                nc.vector.tensor_tensor(
                    eq2[:], eq2[:],
                    w2[:].unsqueeze(2).broadcast_to((P, 4, E)), OP.mult,
                )
                nc.vector.tensor_tensor(cw[:], cw[:], eq2[:], OP.add)

                # expand cw along R, transpose to feature-major, apply, project
                cwt_sb = cwp.tile([P, 4, P], F32, tag="cwt")
                for c in range(4):
                    cw_exp = cwp.tile([P, E, R], F32, tag="cwe")
                    nc.gpsimd.tensor_copy(
                        cw_exp[:],
                        cw[:, c, :].unsqueeze(2).broadcast_to((P, E, R)),
                    )
                    cwt_ps = tp.tile([P, P], F32, tag="tp")
                    nc.tensor.transpose(
                        cwt_ps[:], cw_exp[:].rearrange("p e r -> p (e r)"),
                        ident[:],
                    )
                    nc.scalar.copy(cwt_sb[:, c, :], cwt_ps[:])
                hw_sb = hwp.tile([P, 4, P], F32R)
                nc.vector.tensor_tensor(
                    hw_sb[:].rearrange("p a b -> p (a b)"),
                    h_ps[:],
                    cwt_sb[:].rearrange("p a b -> p (a b)"),
                    OP.mult,
                )
                for pair in range(2):
                    o_sb = osb.tile([P, 2, D], F32, tag="osb")
                    for ci in range(2):
                        c = pair * 2 + ci
                        for h in range(2):
                            o_ps = ops.tile([P, 512], F32, tag="ops")
                            nc.tensor.matmul(
                                o_ps[:],
                                hw_sb[:, c, :],
                                b_r[:, h * 512:(h + 1) * 512],
                                start=True,
                                stop=True,
                            )
                            nc.scalar.copy(
                                o_sb[:, ci, h * 512:(h + 1) * 512], o_ps[:]
                            )
                    nc.sync.dma_start(
                        out_v[s, :, pair * 2:(pair + 1) * 2], o_sb[:]
                    )

    nc.finalize()
    return nc


_NC_CACHE = {}


def _get_nc(n_super=N_SUPER):
    if n_super not in _NC_CACHE:
        _NC_CACHE[n_super] = build(n_super)
    return _NC_CACHE[n_super]


def _prep_weights(Wg, A, B):
    # wg[p, c, e] = Wg[c*128+p, e]
    wg = np.ascontiguousarray(
        Wg.reshape(NCH, P, E).transpose(1, 0, 2)
    ).astype(np.float32)
    # A_flat[d, e*R+r] = A[e, d, r];  a[p, c, j] = A_flat[c*128+p, j]
    a_flat = A.transpose(1, 0, 2).reshape(D, J)
    a = np.ascontiguousarray(
        a_flat.reshape(NCH, P, J).transpose(1, 0, 2)
    ).astype(np.float32)
    # B_flat[j, d] = B[j//R, j%R, d]
    b = np.ascontiguousarray(B.reshape(J, D)).astype(np.float32)
    return wg, a, b


def kernel(x, Wg, A, B):
    x = np.asarray(x, dtype=np.float32)
    orig_shape = x.shape
    xs = np.ascontiguousarray(x.reshape(-1, D))
    assert xs.shape[0] == T_FULL
    wg, a, b = _prep_weights(np.asarray(Wg, np.float32),
                             np.asarray(A, np.float32),
                             np.asarray(B, np.float32))

    nc = _get_nc()
    shards = np.split(xs, N_CORES, axis=0)
    in_maps = [
        {"x": np.ascontiguousarray(sh), "wg": wg, "a": a, "b": b}
        for sh in shards
    ]
    res = run_bass_kernel_spmd(nc, in_maps, list(range(N_CORES)))
    out = np.concatenate([r["out"] for r in res.results], axis=0)
    return out.reshape(orig_shape)


# revision 46
# speedup vs baseline: 204.5066x; 1.0519x over previous
"""MoLoRA (top-2 of 8 LoRA experts, dense compute) Trainium2 Bass kernel.

Math (matches the jax reference exactly in structure):
    xs [T,1024], Wg [1024,8], A_flat [1024,128] (j = e*16+r), B_flat [128,1024]
    logits = xs @ Wg                      (fp32, exact -> exact top-2 choice)
    cw     = dense top-2 softmax weights  [T,8]
    H^T    = A_flat^T @ xs^T              (f32r, feature-major [128 j, T])
    HW^T   = H^T * cw^T(expanded)         (f32r)
    out    = (HW^T)^T-matmul B_flat       (f32r, token-major [T,1024])

Sharding: pure data-parallel over tokens; 8 cores x 4096 tokens.
Per core: 16 pairs x 256 tokens (2 subtiles x 128).  x is transposed
on-chip via PE transposes (fp32); the gate runs in true fp32 off the
transposed x, the expert matmuls run in f32r (fast PE mode, ~1.5e-4
matmul rel-err measured on HW).
"""

import numpy as np

import concourse.bacc as bacc
import concourse.mybir as mybir
import concourse.tile as tile
from concourse.bass_utils import run_bass_kernel_spmd
from concourse.masks import make_identity

F32 = mybir.dt.float32
F32R = mybir.dt.float32r

N_CORES = 8
D = 1024
E = 8
R = 16
J = E * R  # 128
T_FULL = 4 * 8192
T_SH = T_FULL // N_CORES  # 4096
P = 128
NCH = D // P  # 8 contraction chunks
N_PAIR = T_SH // 256  # 16 pairs of 256 tokens
N_SUPER = N_PAIR // 2  # kept for compat with callers
AX = mybir.AxisListType.X
OP = mybir.AluOpType


def build(n_super=N_SUPER, n_reps=1):
    t_sh = n_super * 512
    nc = bacc.Bacc("TRN2", target_bir_lowering=False, debug=False)

    x_d = nc.declare_dram_parameter("x", [t_sh, D], F32, isOutput=False)
    wg_d = nc.declare_dram_parameter("wg", [P, NCH, E], F32, isOutput=False)
    a_d = nc.declare_dram_parameter("a", [P, NCH, J], F32, isOutput=False)
    b_d = nc.declare_dram_parameter("b", [P, D], F32, isOutput=False)
    out_d = nc.declare_dram_parameter("out", [t_sh, D], F32, isOutput=True)

    # s = supertile, c = subtile (4 of 128 tokens), p = token within subtile
    x_v = x_d[:].rearrange("(s c p) d -> s p c d", p=P, c=4)
    out_v = out_d[:].rearrange("(s c p) d -> s p c d", p=P, c=4)

    with tile.TileContext(nc) as tc:
        with (
            tc.tile_pool(name="consts", bufs=1) as consts,
            tc.tile_pool(name="xp", bufs=4) as xp,
            tc.tile_pool(name="xt", bufs=3) as xtp,
            tc.tile_pool(name="xtr", bufs=3) as xtrp,
            tc.tile_pool(name="cwp", bufs=3) as cwp,
            tc.tile_pool(name="hwp", bufs=3) as hwp,
            tc.tile_pool(name="osb", bufs=3) as osb,
            tc.tile_pool(name="tp", bufs=2, space="PSUM") as tp,
            tc.tile_pool(name="gps", bufs=3, space="PSUM") as gps,
            tc.tile_pool(name="hps", bufs=1, space="PSUM") as hps,
            tc.tile_pool(name="ops", bufs=2, space="PSUM") as ops,
        ):
            ident = consts.tile([P, P], F32)
            make_identity(nc, ident[:])
            wg_sb = consts.tile([P, NCH, E], F32)
            a_sb = consts.tile([P, NCH, J], F32)
            b_sb = consts.tile([P, D], F32)
            nc.sync.dma_start(wg_sb[:], wg_d[:])
            nc.sync.dma_start(a_sb[:], a_d[:])
            nc.sync.dma_start(b_sb[:], b_d[:])
            # round expert weights to f32r once
            a_r = consts.tile([P, NCH, J], F32R)
            b_r = consts.tile([P, D], F32R)
            nc.vector.tensor_copy(a_r[:], a_sb[:])
            nc.vector.tensor_copy(b_r[:], b_sb[:])

            def phase_a(s):
                """load + transpose + evacuate + round + gate for supertile s"""
                x_sb = xp.tile([P, 4, D], F32)
                # split the load per subtile so transposes start at 512 KiB
                for c in range(4):
                    nc.sync.dma_start(x_sb[:, c], x_v[s, :, c])

                gate_ps = gps.tile([P, 4, E], F32)
                xt_r = xtrp.tile([P, NCH, 512], F32R)
                for c in range(4):
                    xt_fg0 = xtp.tile([P, 4, P], F32, tag="xt0")
                    xt_fg1 = xtp.tile([P, 4, P], F32, tag="xt1")
                    xt_fg = [xt_fg0, xt_fg1]
                    for g in range(2):
                        tpt = tp.tile([P, 4, P], F32, tag="tp")
                        for k in range(4):
                            kk = g * 4 + k
                            nc.tensor.transpose(
                                tpt[:, k, :],
                                x_sb[:, c, kk * P:(kk + 1) * P],
                                ident[:],
                            )
                        nc.vector.tensor_copy(xt_fg[g][:], tpt[:])
                        # round to f32r for the expert matmuls; split DVE/Pool
                        dst = xt_r[:, g * 4:(g + 1) * 4, c * P:(c + 1) * P]
                        if c < 2:
                            nc.vector.tensor_copy(dst, xt_fg[g][:])
                        else:
                            nc.gpsimd.tensor_copy(dst, xt_fg[g][:])
                        # gate: true-fp32 matmuls, N=8 (stationary = xT chunk)
                        for k in range(4):
                            nc.tensor.matmul(
                                gate_ps[:, c, :],
                                xt_fg[g][:, k, :],
                                wg_sb[:, g * 4 + k, :],
                                start=(g == 0 and k == 0),
                                stop=(g == 1 and k == 3),
                            )
                return gate_ps, xt_r

            def phase_b(s, gate_ps, xt_r):
                """H + top-2 combine + apply + project + store for supertile s"""
                # H^T for this supertile: [128 j, 512 tok], f32r N=512
                h_ps = hps.tile([P, 512], F32)
                for k in range(NCH):
                    nc.tensor.matmul(
                        h_ps[:],
                        a_r[:, k, :],
                        xt_r[:, k, :],
                        start=(k == 0),
                        stop=(k == NCH - 1),
                    )

                # top-2 softmax -> dense combine weights, batched over 4 subtiles
                m1 = cwp.tile([P, 4], F32)
                m2 = cwp.tile([P, 4], F32)
                d21 = cwp.tile([P, 4], F32)
                w1 = cwp.tile([P, 4], F32)
                w2 = cwp.tile([P, 4], F32)
                eq1 = cwp.tile([P, 4, E], F32)
                msk = cwp.tile([P, 4, E], F32)
                eq2 = cwp.tile([P, 4, E], F32)
                cw = cwp.tile([P, 4, E], F32)
                nc.vector.tensor_reduce(m1[:], gate_ps[:], AX, OP.max)
                nc.vector.tensor_tensor(
                    eq1[:], gate_ps[:],
                    m1[:].unsqueeze(2).broadcast_to((P, 4, E)), OP.is_equal,
                )
                nc.vector.scalar_tensor_tensor(
                    msk[:], eq1[:], -1e30, gate_ps[:], OP.mult, OP.add
                )
                nc.vector.tensor_reduce(m2[:], msk[:], AX, OP.max)
                nc.vector.tensor_tensor(d21[:], m2[:], m1[:], OP.subtract)
                nc.scalar.activation(
                    w1[:], d21[:], mybir.ActivationFunctionType.Sigmoid, scale=-1.0
                )
                nc.scalar.activation(
                    w2[:], d21[:], mybir.ActivationFunctionType.Sigmoid
                )
                nc.vector.tensor_tensor(
                    eq2[:], msk[:],
                    m2[:].unsqueeze(2).broadcast_to((P, 4, E)), OP.is_equal,
                )
                nc.vector.tensor_tensor(
                    cw[:], eq1[:],
                    w1[:].unsqueeze(2).broadcast_to((P, 4, E)), OP.mult,
                )
                nc.vector.tensor_tensor(
                    eq2[:], eq2[:],
                    w2[:].unsqueeze(2).broadcast_to((P, 4, E)), OP.mult,
                )
                nc.vector.tensor_tensor(cw[:], cw[:], eq2[:], OP.add)

                # expand cw along R, transpose to feature-major, apply, project
                cwt_sb = cwp.tile([P, 4, P], F32, tag="cwt")
                for c in range(4):
                    cw_exp = cwp.tile([P, E, R], F32, tag="cwe")
                    nc.gpsimd.tensor_copy(
                        cw_exp[:],
                        cw[:, c, :].unsqueeze(2).broadcast_to((P, E, R)),
                    )
                    cwt_ps = tp.tile([P, P], F32, tag="tp")
                    nc.tensor.transpose(
                        cwt_ps[:], cw_exp[:].rearrange("p e r -> p (e r)"),
                        ident[:],
                    )
                    nc.scalar.copy(cwt_sb[:, c, :], cwt_ps[:])
                hw_sb = hwp.tile([P, 4, P], F32R)
                nc.vector.tensor_tensor(
                    hw_sb[:].rearrange("p a b -> p (a b)"),
                    h_ps[:],
                    cwt_sb[:].rearrange("p a b -> p (a b)"),
                    OP.mult,
                )
                for pair in range(2):
                    o_sb = osb.tile([P, 2, D], F32, tag="osb")
                    for ci in range(2):
                        c = pair * 2 + ci
                        for h in range(2):
                            o_ps = ops.tile([P, 512], F32, tag="ops")
                            nc.tensor.matmul(
                                o_ps[:],
                                hw_sb[:, c, :],
                                b_r[:, h * 512:(h + 1) * 512],
                                start=True,
                                stop=True,
                            )
                            nc.scalar.copy(
                                o_sb[:, ci, h * 512:(h + 1) * 512], o_ps[:]
                            )
                    nc.sync.dma_start(
                        out_v[s, :, pair * 2:(pair + 1) * 2], o_sb[:]
                    )

            # 1-stage software pipeline: emit A(s+1) before B(s) so the PE's
            # in-order queue has s+1's transposes ahead of s's tail matmuls
            # (H waits on the f32r rounds; without the skew the PE stalls
            # there while ready transpose work sits behind it).
            order = [t % n_super for t in range(n_super * n_reps)]
            from collections import deque
            pend = deque()
            for s in order:
                a = phase_a(s)
                pend.append((s, *a))
                if len(pend) > 2:
                    phase_b(*pend.popleft())
            while pend:
                phase_b(*pend.popleft())

    nc.finalize()
    return nc


_NC_CACHE = {}


def _get_nc(n_super=N_SUPER):
    if n_super not in _NC_CACHE:
        _NC_CACHE[n_super] = build(n_super)
    return _NC_CACHE[n_super]


def _prep_weights(Wg, A, B):
    # wg[p, c, e] = Wg[c*128+p, e]
    wg = np.ascontiguousarray(
        Wg.reshape(NCH, P, E).transpose(1, 0, 2)
    ).astype(np.float32)
    # A_flat[d, e*R+r] = A[e, d, r];  a[p, c, j] = A_flat[c*128+p, j]
    a_flat = A.transpose(1, 0, 2).reshape(D, J)
    a = np.ascontiguousarray(
        a_flat.reshape(NCH, P, J).transpose(1, 0, 2)
    ).astype(np.float32)
    # B_flat[j, d] = B[j//R, j%R, d]
    b = np.ascontiguousarray(B.reshape(J, D)).astype(np.float32)
    return wg, a, b


def kernel(x, Wg, A, B):
    x = np.asarray(x, dtype=np.float32)
    orig_shape = x.shape
    xs = np.ascontiguousarray(x.reshape(-1, D))
    assert xs.shape[0] == T_FULL
    wg, a, b = _prep_weights(np.asarray(Wg, np.float32),
                             np.asarray(A, np.float32),
                             np.asarray(B, np.float32))

    nc = _get_nc()
    shards = np.split(xs, N_CORES, axis=0)
    in_maps = [
        {"x": np.ascontiguousarray(sh), "wg": wg, "a": a, "b": b}
        for sh in shards
    ]
    res = run_bass_kernel_spmd(nc, in_maps, list(range(N_CORES)))
    out = np.concatenate([r["out"] for r in res.results], axis=0)
    return out.reshape(orig_shape)
